# revision 1
# baseline (speedup 1.0000x reference)
"""Trainium2 Bass kernel for nn_DTAM (differential transposed-attention module).

Sharding: 8 cores = batch(4) x head(2). Each core computes its (b, h) shard
end-to-end; host does LayerNorm scale precompute, weight folding, and the
final partial-sum + residual merge.

v2 design: the pointwise C->2C conv and the depthwise 3x3 are FUSED into a
single dense 3x3 conv (kernel K[o,c,dy,dx] = D[o,dy,dx] * W[o,c]) executed
as 9 tap-matmuls on PE in fp8e4m3 with DoubleRow perf mode (2 contraction
k-tiles = the two 96-channel halves, 0.5 cycles/row). xs stays fully
resident in SBUF as fp8 with zero-padded boundary rows, so there are no
supertiles and no halo copies. Power-of-2 scales folded into the fp8
weights are compensated exactly in the softmax temperature and the RMS
epsilon.

Per core:
  phase A: per 8-row chunk, 9 DoubleRow tap-matmuls accumulate the conv in
           PSUM for each of q/k/v x half; evac f32->bf16 (q on ACT, k on
           DVE, v on GPSIMD); per 16-row super: DMA-xbar transpose of
           dwq/dwk (bf16, issued from SP), then channel-attention score
           matmuls (PE, bf16) accumulated into SBUF.
  phase B: scale scores by t/(sq*sk), softmax halves, attn = a1 - lam*a2,
           PE transpose of attn.
  phase C: y = attnT @ dwv (PE bf16), RMS stats (ones-matmul bf16),
           r = rsqrt(mean+eps') (ACT), r broadcast via K=1 bf16 matmul,
           out proj (PE), evac*r (DVE), DMA out.
"""

import numpy as np
import ml_dtypes
from contextlib import ExitStack

BF16 = ml_dtypes.bfloat16
FP8 = ml_dtypes.float8_e4m3fn

# ---- problem constants (hardcoded per contest rules) ----
B, C, H, W = 4, 192, 128, 128
HEADS = 2
N = H * W                 # 16384
HC = 96                   # half-channels per head (q1/q2 split)
LAM_INIT = 0.8
NSUP = 8                  # supers (16 rows each) for transpose/score blocks
ROWS = 16
SUP = ROWS * W            # 2048 px
RC = 8                    # conv row-chunk (PSUM tile rows)
CH = 512                  # phase C chunk

_CACHED = {}


def _build_program():
    import concourse.bass as bass
    import concourse.bacc as bacc
    import concourse.tile as tile
    from concourse import mybir

    f32 = mybir.dt.float32
    bf16 = mybir.dt.bfloat16
    fp8 = mybir.dt.float8e4
    AF = mybir.ActivationFunctionType
    OP = mybir.AluOpType
    AX = mybir.AxisListType
    DR = mybir.MatmulPerfMode.DoubleRow

    nc = bacc.Bacc("TRN2", target_bir_lowering=False, debug=False,
                   num_devices=8)

    # ---- DRAM I/O ----
    # xs fp8, channel halves interleaved, rows 0/129 zero-padded:
    # xs8[c, j, 1+y, x] = xs[c + 96*j, y, x]
    XJ = (H + 2) * W
    xs_d = nc.dram_tensor("xs8", [96, 2 * XJ], fp8, kind="ExternalInput")
    # fused conv weights per tensor/half: [96(c), 9(t), 2(j), 96(o)]
    w_d = {}
    for p in ("q", "k", "v"):
        for hf in range(2):
            w_d[(p, hf)] = nc.dram_tensor(
                f"w{p}{hf}", [96, 9 * 2 * 96], fp8, kind="ExternalInput")
    wo_1 = nc.dram_tensor("wo_1", [96, 192], bf16, kind="ExternalInput")
    wo_2 = nc.dram_tensor("wo_2", [96, 192], bf16, kind="ExternalInput")
    ones96_d = nc.dram_tensor("ones96", [96, 1], bf16, kind="ExternalInput")
    ones1_d = nc.dram_tensor("ones1", [1, 128], bf16, kind="ExternalInput")
    ident_d = nc.dram_tensor("ident", [96, 96], bf16, kind="ExternalInput")
    neglam_d = nc.dram_tensor("neglam", [128, 1], f32, kind="ExternalInput")
    tsc_d = nc.dram_tensor("tsc", [96, 2], f32, kind="ExternalInput")
    epsd_d = nc.dram_tensor("epsd", [1, 1], f32, kind="ExternalInput")
    out_d = nc.dram_tensor("out", [192, N], f32, kind="ExternalOutput")

    # tap t in 0..8 -> spatial offset (oy, ox), correlation convention
    OFFS = [(t // 3 - 1, t % 3 - 1) for t in range(9)]
    TAP_ORDER = [4] + [t for t in range(9) if t != 4]

    def xr(ox):
        if ox == -1:
            return (1, 128), (0, 127)
        if ox == 1:
            return (0, 127), (1, 128)
        return (0, 128), (0, 128)

    with tile.TileContext(nc) as tc, ExitStack() as ctx:
        cst = ctx.enter_context(tc.tile_pool(name="cst", bufs=1))
        res = ctx.enter_context(tc.tile_pool(name="res", bufs=1))

        # ---- load constants ----
        xs8 = cst.tile([96, 2, H + 2, W], fp8, name="xs8", tag="xs8")
        nc.sync.dma_start(xs8[:].rearrange("p a b c -> p (a b c)"), xs_d[:])
        wt = {}
        for p in ("q", "k", "v"):
            for hf in range(2):
                t = cst.tile([96, 9, 2, 96], fp8, name=f"w{p}{hf}",
                             tag=f"w{p}{hf}")
                nc.sync.dma_start(t[:].rearrange("p a b c -> p (a b c)"),
                                  w_d[(p, hf)][:])
                wt[(p, hf)] = t
        wo1 = cst.tile([96, 192], bf16, name="wo1", tag="wo1")
        wo2 = cst.tile([96, 192], bf16, name="wo2", tag="wo2")
        nc.sync.dma_start(wo1[:], wo_1[:])
        nc.sync.dma_start(wo2[:], wo_2[:])
        ones96 = cst.tile([96, 1], bf16, name="o96", tag="o96")
        ones1 = cst.tile([1, 128], bf16, name="o1", tag="o1")
        ident = cst.tile([96, 96], bf16, name="id", tag="id")
        neglam = cst.tile([128, 1], f32, name="nl", tag="nl")
        tsc = cst.tile([96, 2], f32, name="tsc", tag="tsc")
        epsd = cst.tile([1, 1], f32, name="epsd", tag="epsd")
        nc.sync.dma_start(ones96[:], ones96_d[:])
        nc.sync.dma_start(ones1[:], ones1_d[:])
        nc.sync.dma_start(ident[:], ident_d[:])
        nc.sync.dma_start(neglam[:], neglam_d[:])
        nc.sync.dma_start(tsc[:], tsc_d[:])
        nc.sync.dma_start(epsd[:], epsd_d[:])

        # resident dwv halves (bf16)
        dwv_res = [res.tile([96, N], bf16, name=f"dwv{i}", tag=f"dwv{i}")
                   for i in range(2)]

        smx = ctx.enter_context(tc.tile_pool(name="smx", bufs=1))
        # SBUF score accumulators (summed over supers)
        sc = [res.tile([96, 96], f32, name=f"sc{i}", tag=f"sc{i}")
              for i in range(2)]
        nc.vector.memset(sc[0][:], 0.0)
        nc.vector.memset(sc[1][:], 0.0)

        # ================= PHASE A =================
        with tc.tile_pool(name="dwo", bufs=2) as dwo, \
             tc.tile_pool(name="tro", bufs=2) as tro, \
             tc.tile_pool(name="cvps", bufs=2, space="PSUM") as cvps, \
             tc.tile_pool(name="scps", bufs=2, space="PSUM") as scps_p:

            for s in range(NSUP):
                dwqk = {}
                for p in ("q", "k"):
                    for hf in range(2):
                        dwqk[(p, hf)] = dwo.tile(
                            [96, ROWS, 128], bf16, name=f"dw{p}{hf}",
                            tag=f"dw{p}{hf}")
                for p in ("q", "k", "v"):
                    for hf in range(2):
                        for rc in range(2):
                            r0 = s * ROWS + rc * RC     # image row of chunk
                            ps = cvps.tile([96, RC, 128], f32, name="cv",
                                           tag="cv")
                            # moving AP is capped at 512 elements, so the
                            # 9-tap DoubleRow chains run per 2-row group
                            for qc in range(RC // 2):
                                q0 = r0 + qc * 2
                                for ti, t in enumerate(TAP_ORDER):
                                    oy, ox = OFFS[t]
                                    (a0, a1), (b0, b1) = xr(ox)
                                    nc.tensor.matmul(
                                        ps[:, qc * 2:qc * 2 + 2, a0:a1],
                                        wt[(p, hf)][:, t, :, :],
                                        xs8[:, :, 1 + q0 + oy:3 + q0 + oy,
                                            b0:b1],
                                        start=(ti == 0), stop=(ti == 8),
                                        perf_mode=DR,
                                        skip_group_check=True)
                            src = ps[:]
                            if p == "v":
                                seg = r0 * W
                                dst = dwv_res[hf][:, seg:seg + RC * W]\
                                    .rearrange("p (r x) -> p r x", x=128)
                                if hf == 0:
                                    nc.scalar.copy(dst, src)
                                else:
                                    nc.vector.tensor_copy(dst, src)
                            elif p == "q":
                                nc.scalar.copy(
                                    dwqk[(p, hf)][:, rc * RC:rc * RC + RC, :],
                                    src)
                            else:
                                nc.vector.tensor_copy(
                                    dwqk[(p, hf)][:, rc * RC:rc * RC + RC, :],
                                    src)
                # DMA-xbar transposes of dwq/dwk -> [128, 16, 96] (issue on SP)
                trt = {}
                for p in ("q", "k"):
                    for hf in range(2):
                        tt = tro.tile([128, ROWS, 96], bf16,
                                      name=f"t{p}{hf}", tag=f"t{p}{hf}")
                        trt[(p, hf)] = tt
                        nc.sync.dma_start_transpose(
                            tt[:], dwqk[(p, hf)][:].rearrange(
                                "p r x -> p (r x)"))
                # score matmuls (accumulate in PSUM per super, then fold into
                # the SBUF accumulator)
                for hf in range(2):
                    psc = scps_p.tile([96, 96], f32, name=f"psc{hf}",
                                      tag="psc")
                    for blk in range(ROWS):
                        nc.tensor.matmul(
                            psc[:],
                            trt[("q", hf)][:, blk, :],
                            trt[("k", hf)][:, blk, :],
                            start=(blk == 0),
                            stop=(blk == ROWS - 1))
                    nc.vector.tensor_tensor(sc[hf][:], sc[hf][:], psc[:],
                                            OP.add)

        # ================= PHASE B: softmax + attn =================
        atstack = ExitStack()
        atps = atstack.enter_context(
            tc.tile_pool(name="atps", bufs=1, space="PSUM"))
        ex = []
        rr_ = []
        for hf in range(2):
            scl = smx.tile([96, 96], f32, name=f"scl{hf}", tag=f"scl{hf}")
            nc.vector.tensor_scalar(scl[:], sc[hf][:], tsc[:, hf:hf + 1],
                                    None, OP.mult)
            nm = smx.tile([96, 1], f32, name=f"nm{hf}", tag=f"nm{hf}")
            nc.vector.tensor_reduce(nm[:], scl[:], AX.X, OP.max, negate=True)
            e = smx.tile([96, 96], f32, name=f"e{hf}", tag=f"e{hf}")
            nc.scalar.activation(e[:], scl[:], AF.Exp, bias=nm[:, 0:1])
            sm = smx.tile([96, 1], f32, name=f"sm{hf}", tag=f"sm{hf}")
            nc.vector.tensor_reduce(sm[:], e[:], AX.X, OP.add)
            r = smx.tile([96, 1], f32, name=f"r{hf}", tag=f"r{hf}")
            nc.vector.reciprocal(r[:], sm[:])
            ex.append(e)
            rr_.append(r)
        r2n = smx.tile([96, 1], f32, name="r2n", tag="r2n")
        nc.vector.tensor_scalar(r2n[:], rr_[1][:], neglam[0:96, 0:1],
                                None, OP.mult)
        a1 = smx.tile([96, 96], f32, name="a1", tag="a1")
        nc.scalar.mul(a1[:], ex[0][:], rr_[0][:, 0:1])
        attn = smx.tile([96, 96], bf16, name="attn", tag="attn")
        nc.vector.scalar_tensor_tensor(attn[:], ex[1][:], r2n[:, 0:1],
                                       a1[:], OP.mult, OP.add)
        pt = atps.tile([96, 96], bf16, name="pt", tag="pt")
        nc.tensor.transpose(pt[:], attn[:], ident[:])
        attnT = smx.tile([96, 96], bf16, name="attnT", tag="attnT")
        nc.scalar.copy(attnT[:], pt[:])
        atstack.close()

        # ================= PHASE C =================
        with tc.tile_pool(name="yp", bufs=2) as yp, \
             tc.tile_pool(name="op_", bufs=2) as op_, \
             tc.tile_pool(name="yps", bufs=2, space="PSUM") as yps, \
             tc.tile_pool(name="sqps", bufs=1, space="PSUM") as sqps, \
             tc.tile_pool(name="rbps", bufs=1, space="PSUM") as rbps, \
             tc.tile_pool(name="ops", bufs=2, space="PSUM") as ops:
            for cc in range(N // CH):
                seg = cc * CH
                ysb = []
                yyb = []
                for hf in range(2):
                    py = yps.tile([96, CH], f32, name=f"y{hf}", tag=f"y{hf}")
                    nc.tensor.matmul(py[:], attnT[:],
                                     dwv_res[hf][:, seg:seg + CH],
                                     start=True, stop=True)
                    ys = yp.tile([96, CH], bf16, name=f"ys{hf}",
                                 tag=f"ys{hf}")
                    nc.scalar.copy(ys[:], py[:])
                    yy = yp.tile([96, CH], bf16, name=f"yy{hf}",
                                 tag=f"yy{hf}")
                    nc.gpsimd.tensor_tensor(yy[:], ys[:], ys[:], OP.mult)
                    ysb.append(ys)
                    yyb.append(yy)
                pss = sqps.tile([1, CH], f32, name="ss", tag="ss")
                nc.tensor.matmul(pss[:], ones96[:], yyb[0][:],
                                 start=True, stop=False)
                nc.tensor.matmul(pss[:], ones96[:], yyb[1][:],
                                 start=False, stop=True)
                rsb = op_.tile([1, CH], bf16, name="rs", tag="rs")
                nc.scalar.activation(rsb[:], pss[:],
                                     AF.Abs_reciprocal_sqrt,
                                     bias=epsd[0:1, 0:1],
                                     scale=1.0 / 192.0)
                prb = rbps.tile([128, CH], f32, name="rb", tag="rb")
                nc.tensor.matmul(prb[:], ones1[:], rsb[:],
                                 start=True, stop=True)
                rbsb = op_.tile([128, CH], f32, name="rbs", tag="rbs")
                nc.vector.tensor_copy(rbsb[:], prb[:])
                for mt in range(2):
                    po = ops.tile([96, CH], f32, name="po", tag="po")
                    nc.tensor.matmul(po[:], wo1[:, mt * 96:(mt + 1) * 96],
                                     ysb[0][:], start=True, stop=False)
                    nc.tensor.matmul(po[:], wo2[:, mt * 96:(mt + 1) * 96],
                                     ysb[1][:], start=False, stop=True)
                    osb = op_.tile([96, CH], f32, name=f"os{mt}",
                                   tag=f"os{mt}")
                    nc.vector.tensor_tensor(osb[:], po[:],
                                            rbsb[0:96, :], OP.mult)
                    nc.sync.dma_start(
                        out_d[mt * 96:(mt + 1) * 96, seg:seg + CH],
                        osb[:])
    nc.compile()
    return nc


def _pow2_scale(maxabs, target=128.0):
    if maxabs <= 0:
        return 1.0
    return float(2.0 ** np.floor(np.log2(target / maxabs)))


def _prep_inputs(inputs):
    x = np.asarray(inputs["x"], np.float32)
    norm_w = np.asarray(inputs["norm_w"], np.float32)
    Wq = np.asarray(inputs["Wq"], np.float32)
    Wk = np.asarray(inputs["Wk"], np.float32)
    Wv = np.asarray(inputs["Wv"], np.float32)
    Dq = np.asarray(inputs["Dq"], np.float32)
    Dk = np.asarray(inputs["Dk"], np.float32)
    Dv = np.asarray(inputs["Dv"], np.float32)
    t1 = np.asarray(inputs["t1"], np.float32)
    t2 = np.asarray(inputs["t2"], np.float32)
    hn_w = np.asarray(inputs["hn_w"], np.float32)
    Wo = np.asarray(inputs["Wo"], np.float32)
    lam = float(np.exp(np.sum(inputs["lq1"] * inputs["lk1"],
                              dtype=np.float64))
                - np.exp(np.sum(inputs["lq2"] * inputs["lk2"],
                                dtype=np.float64))
                + LAM_INIT)

    # LayerNorm scale on host
    var = x.var(axis=1)                       # [B, H, W]
    s = 1.0 / np.sqrt(var + 1e-5)
    xs = (x * s[:, None, :, :]).reshape(B, C, H, W)

    Wq_f = Wq * norm_w[None, :]
    Wk_f = Wk * norm_w[None, :]
    Wv_f = Wv * norm_w[None, :]

    in_maps = []
    for core in range(8):
        b, h = core // 2, core % 2
        sl = slice(h * 192, (h + 1) * 192)
        m = {}
        # xs fp8 interleaved with zero-padded rows
        xpad = np.zeros((96, 2, H + 2, W), np.float32)
        xc = xs[b]                            # [192, H, W]
        xpad[:, 0, 1:H + 1, :] = xc[0:96]
        xpad[:, 1, 1:H + 1, :] = xc[96:192]
        m["xs8"] = np.clip(xpad, -224, 224).astype(FP8).reshape(96, -1)

        scales = {}
        for nm, Wf, Dd in (("q", Wq_f, Dq), ("k", Wk_f, Dk),
                           ("v", Wv_f, Dv)):
            Wh = Wf[sl]                       # [192 out, 192 in]
            dh = Dd[sl, 0].reshape(192, 9)    # [192 out, 9 taps]
            # K3[o, c, t] = Wh[o, c] * dh[o, t]
            K3 = Wh[:, :, None] * dh[:, None, :]
            sp = _pow2_scale(np.abs(K3).max())
            scales[nm] = sp
            K3s = np.clip(K3 * sp, -224, 224)
            for hf in range(2):
                # w8[c, t, j, o] = K3s[hf*96+o, c+96j, t]
                blk = K3s[hf * 96:(hf + 1) * 96]   # [96 o, 192 cg, 9 t]
                w8 = blk.transpose(1, 2, 0).reshape(2, 96, 9, 96)
                # [192 cg, 9 t, 96 o] -> split cg=(j, c): [2 j, 96 c, 9 t, 96 o]
                m[f"w{nm}{hf}"] = np.ascontiguousarray(
                    w8.transpose(1, 2, 0, 3)   # [96 c, 9 t, 2 j, 96 o]
                ).astype(FP8).reshape(96, -1)

        th = np.array([t1[h, 0, 0], t2[h, 0, 0]], np.float32)
        m["tsc"] = np.broadcast_to(
            (th / (scales["q"] * scales["k"]))[None, :], (96, 2)
        ).astype(np.float32).copy()
        m["epsd"] = np.full((1, 1), 1e-6 * scales["v"] ** 2, np.float32)

        Wo_hf = Wo[:, sl] * (hn_w[h] * (1.0 - LAM_INIT))[None, :]
        lhsT = Wo_hf.T.astype(BF16)           # [192 y-ch, 192 out]
        m["wo_1"] = np.ascontiguousarray(lhsT[0:96])
        m["wo_2"] = np.ascontiguousarray(lhsT[96:192])
        m["ones96"] = np.ones((96, 1), BF16)
        m["ones1"] = np.ones((1, 128), BF16)
        m["ident"] = np.eye(96, dtype=BF16)
        m["neglam"] = np.full((128, 1), -lam, np.float32)
        in_maps.append(m)
    return in_maps


def kernel(**inputs):
    from concourse import bass_utils

    if "nc" not in _CACHED:
        _CACHED["nc"] = _build_program()
    nc = _CACHED["nc"]

    in_maps = _prep_inputs(inputs)
    results = bass_utils.run_bass_kernel_spmd(
        nc, in_maps, core_ids=list(range(8))).results

    x = np.asarray(inputs["x"], np.float32)
    out = np.empty((B, C, N), np.float32)
    for b in range(B):
        out[b] = results[2 * b]["out"] + results[2 * b + 1]["out"]
    out = out.reshape(B, C, H, W) + x
    return out.astype(np.float32)

